# revision 30
# baseline (speedup 1.0000x reference)
"""DoubleAttention forward on 8 Trainium2 NeuronCores.

Reference (per sample, x: [512, 4096] after flattening h*w):
    A = wA @ x + bA            [128, n]
    B = wB @ x + bB            [128, n]
    V = wV @ x + bV            [128, n]
    M = softmax(B, axis=ch)    [128, n]
    W = softmax(V, axis=ch)    [128, n]
    gd = A @ M.T               [128, 128]
    Z = gd @ W                 [128, n]
    out = wR @ Z + bR          [512, n]

Sharding: data-parallel over batch, 16 samples -> 8 cores x 2 each.

Implementation notes:
  - All matmul inputs fp16 (exact products, fp32 PSUM accumulation);
    x / weights converted host-side, which also halves the input DMA.
    (fp8 was measured 6-9e-2 end-to-end rel err -- over the 2e-2 gate --
    because the channel softmax exponentially amplifies logit error.)
  - Transposed layout: per 128-wide n-tile, P1[n, A|B|V] = x_chunk.T @
    [wA.T|wB.T|wV.T], so the channel softmax is a free-dim op.
  - n-tiles processed in PAIRS sharing one 2-bank PSUM tile so the
    elementwise ops run at 2x width (amortizes fixed per-op cost).
  - Softmax normalization is NOT materialized: exp writes
    exp(logit - 12*ln2) fp16 straight into the mwall store; A evacuates
    as a PLAIN cast immediately after the projection matmuls (releasing
    the 2-bank PSUM pair tile without waiting on the reduce chain), and
    the 1/sum factors are folded into the M and W halves in place.
  - mwall layout [128, bv, pair, j, 128] keeps the whole exp(V) plane of
    a sample CONTIGUOUS, so W^T tiles for each output group come from
    ONE hardware DMA-transpose (XBAR) issued on the otherwise idle SP
    queue -- no PE transposes, no pW PSUM pool, no DVE evacuation of the
    transposed tiles.  (Verified on HW: dma transpose [128,512]->
    [128,4,128] gives out[p,t,q] = in[q,128t+p], exactly W^T per n-tile.)
  - Elementwise work is SPLIT ACROSS ACT/DVE/POOL: per pair, ACT does
    exp + A-cast, DVE does reduce + recip + M-norm (keeping the
    gd-critical chain on one engine), POOL does W-norm.  Output
    evacuations go ACT(k0,k1)/DVE(k2)/POOL(k3) for the interleaved
    sample and ACT-wide(k0k1 via idle pP)/DVE(k2,k3) for the tail
    sample; output stores issue from POOL's software DGE so the
    hardware DGEs stay free for loads/transposes.
  - gd accumulates on PE across tiles (4-pair emission lag gives the
    cross-engine softmax chain ~6us of slack before the PE needs it);
    both samples' gd accumulators share one PSUM bank ([128, 2, 128]).
  - out = wR @ (gd @ W) is reassociated to (wR @ gd) @ W: G^T = gd.T@wR.T
    is one N=512 matmul per sample, so phase 3 is just four direct
    output matmuls per group feeding DMA'd W^T tiles (no Z round-trip).
  - DMA descriptor shaping: descriptor GENERATION (~0.65us per dma_start
    on the issuing engine) is the load-issue bottleneck, so only the
    first input slice is split fine (per-k 512-col pieces, the first two
    issued from the Scalar HWDGE in parallel with wcat on SP); later
    slices are single whole-slice transfers.  wcat is host-packed to one
    contiguous 3KB line per partition.
  - SAMPLE INTERLEAVE: sample 0's phase 3 (evacuation-bound) is emitted
    interleaved with sample 1's phase 1 (PE-bound); sample 1's first two
    pairs are emitted BEFORE sample 0's gd tail so the PE has work while
    the tail softmax chain drains.
  - Sample 1's output stores are per-group (512 cols) and the last two
    groups store per-k pieces the moment each evacuation lands,
    alternating SP/POOL issuers, to shorten the final store drain (the
    kernel tail is write-bandwidth-bound: 4.2MB of sample-1 output can
    only exist after sample-1's gd completes).
  - Output staged fp16 and upcast host-side (halves the store DMA).
  - Biases fold in as rank-1 PSUM-accumulate matmuls / ACT bias adds,
    all skipped when the bias vectors are zero (the common case).
"""

import sys

if "/opt/trn_rl_repo" not in sys.path:
    sys.path.insert(0, "/opt/trn_rl_repo")

import numpy as np

import concourse.bacc as bacc
import concourse.tile as tile
from concourse import masks, mybir
from concourse.bass_utils import run_bass_kernel_spmd

N_CORES = 8
B_GLOBAL = 16
B_LOC = B_GLOBAL // N_CORES
C_IN, C_M, C_N = 512, 128, 128
H = W = 64
N = H * W                      # 4096 spatial positions
NT = N // 128                  # 32 tiles of 128 positions
NP = NT // 2                   # 16 tile-pairs
KC = C_IN // 128               # 4 contraction chunks
NE = N // 1024                 # 4 x-load slices of 1024 columns
NG = N // 512                  # 8 output groups of 512 positions
SHIFT = float(-12.0 * np.log(2.0))   # exp downshift so fp16 never overflows
F32 = mybir.dt.float32
F16 = mybir.dt.float16
EXP = mybir.ActivationFunctionType.Exp
IDENT = mybir.ActivationFunctionType.Identity


def _build(has_bias_abv: bool, has_bias_r: bool):
    nc = bacc.Bacc("TRN2", target_bir_lowering=False, debug=False)

    x_d = nc.dram_tensor("x", (B_LOC, 128, NE, KC, 1024), F16,
                     kind="ExternalInput")
    wcat_d = nc.dram_tensor("wcat", (128, KC * 384), F16, kind="ExternalInput")
    wrt_d = nc.dram_tensor("wrt", (128, C_IN), F16, kind="ExternalInput")
    if has_bias_abv:
        bcat_d = nc.dram_tensor("bcat", (1, 384), F16, kind="ExternalInput")
    if has_bias_r:
        brt_d = nc.dram_tensor("brt", (128, KC), F32, kind="ExternalInput")
    out_d = nc.dram_tensor("out", (B_LOC, 128, NG, KC, 512), F16,
                       kind="ExternalOutput")

    with tile.TileContext(nc) as tc:
        with (
            tc.tile_pool(name="const", bufs=1) as constp,
            tc.tile_pool(name="xq", bufs=B_LOC * NE) as xqp,
            tc.tile_pool(name="mw", bufs=B_LOC) as mwp,
            tc.tile_pool(name="at", bufs=7) as atp,
            tc.tile_pool(name="st", bufs=4) as stp,
            tc.tile_pool(name="gds", bufs=2) as gdsp,
            tc.tile_pool(name="gts", bufs=2) as gtsp,
            tc.tile_pool(name="wsb", bufs=B_LOC * NG) as wsbp,
            tc.tile_pool(name="osb", bufs=3) as osbp,
            tc.tile_pool(name="pP", bufs=2, space="PSUM") as pP,
            tc.tile_pool(name="pG", bufs=1, space="PSUM") as pG,
            tc.tile_pool(name="pW", bufs=1, space="PSUM") as pW,
            tc.tile_pool(name="pO", bufs=2, space="PSUM") as pO,
        ):
            # wcat first: it gates every phase-1 matmul.  Host-packed to one
            # contiguous 3 KB line per partition -> only 128 descriptors.
            # wcat k0-block first: pair 0's first matmul needs only
            # wcat[:, 0, :] -- landing it ~1us earlier starts the pipeline
            # that much sooner
            wcat = constp.tile([128, KC, 384], F16)
            wcat_v = wcat_d.ap().rearrange("p (k j) -> p k j", k=KC)
            nc.sync.dma_start(wcat[:, 0:1, :], wcat_v[:, 0:1, :])
            nc.sync.dma_start(wcat[:, 1:KC, :], wcat_v[:, 1:KC, :])

            # x prefetch.  x is HOST-PACKED to [128, NE, KC, 1024]: one
            # whole slice is 8KB contiguous per partition = one dma_start
            # with 128 fat descriptors (~0.65us descriptor-gen).  Only the
            # first slice is split per-k for the pipeline ramp, with k0/k1
            # issued from the Scalar HWDGE (its queue is free before the
            # first exp) IN PARALLEL with wcat on SP.
            xqs = []
            for s in range(B_LOC):
                xq = [
                    xqp.tile([128, KC, 1024], F16, tag="xq", name=f"xq{s}_{e}")
                    for e in range(NE)
                ]
                src = x_d[s]
                if s == 0:
                    # pair-0 data first: 32KB pieces land in ~0.3us; the
                    # remainder of slice 0 as per-k pieces, slice 1 in
                    # halves (ring round-robin makes everything issued-
                    # so-far complete together, so granularity = latency)
                    for k in (0, 1):
                        nc.scalar.dma_start(
                            xq[0][:, k:k + 1, 0:128], src[:, 0, k:k + 1, 0:128]
                        )
                    for k in (2, 3):
                        nc.sync.dma_start(
                            xq[0][:, k:k + 1, 0:128], src[:, 0, k:k + 1, 0:128]
                        )
                    for k in (0, 1):
                        nc.scalar.dma_start(
                            xq[0][:, k:k + 1, 128:1024],
                            src[:, 0, k:k + 1, 128:1024],
                        )
                    for k in (2, 3):
                        nc.sync.dma_start(
                            xq[0][:, k:k + 1, 128:1024],
                            src[:, 0, k:k + 1, 128:1024],
                        )
                    for h in (0, 1):
                        nc.sync.dma_start(
                            xq[1][:, :, h * 512:(h + 1) * 512],
                            src[:, 1, :, h * 512:(h + 1) * 512],
                        )
                    for e in range(2, NE):
                        nc.sync.dma_start(xq[e][:], src[:, e])
                else:
                    for e in range(NE):
                        nc.sync.dma_start(xq[e][:], src[:, e])
                xqs.append(xq)

            wrt = constp.tile([128, C_IN], F16)
            nc.sync.dma_start(wrt[:], wrt_d[:])
            ident16 = constp.tile([128, 128], F16)
            masks.make_identity(nc, ident16[:])
            # PE warmup: a SHORT burst of wide full-duty matmuls ending
            # right when the first pair's data lands, so the HAM clock gate
            # sees sustained busy and lifts the 1.2GHz cold clock early.
            # (A LATE or long-drawn warmup measurably backfires -- runs
            # whose 10-17us window shows mediocre PE duty settle at 2.0GHz
            # for the WHOLE kernel.)
            warmsrc = constp.tile([128, 512], F16)
            nc.vector.memset(warmsrc[:], 0.5)
            warmt = pO.tile([128, 512], F32, tag="po", name="warmup")
            for w in range(8):
                nc.tensor.matmul(
                    warmt[:], ident16[:], warmsrc[:],
                    start=True, stop=True, skip_group_check=True,
                )
            shift = constp.tile([128, 1], F32)
            nc.gpsimd.memset(shift[:], SHIFT)
            if has_bias_abv:
                bcat = constp.tile([1, 384], F16)
                nc.sync.dma_start(bcat[:], bcat_d[:])
                ones1 = constp.tile([1, 128], F16)
                nc.gpsimd.memset(ones1[:], 1.0)
            if has_bias_r:
                brt = constp.tile([128, KC], F32)
                nc.sync.dma_start(brt[:], brt_d[:])

            # per-sample pipeline state
            # mwall[s]: [128, bv(2), pair(16), j(2), 128] -- B plane then V
            # plane; the V plane is contiguous so group W^T tiles come from
            # one DMA transpose each.
            mwall = {}
            ats = {}    # (s, i) -> A fp16 [128, 2, 128]
            gdt = None  # shared [128, 2, 128] PSUM accumulator (s on mid dim)
            gts = {}    # s -> G^T fp16 [128, 512]
            wsbs = {}   # (s, g) -> W^T fp16 [128, 4, 128]

            def emit_gd(s, i, first, last):
                # gd[m,k] += sum_n (A[m,n]/sB[n]) * expB[k,n]
                for j in (0, 1):
                    nc.tensor.matmul(
                        gdt[:, s, :], ats[s, i][:, j, :],
                        mwall[s][:, 0, i, j, :],
                        start=(first and j == 0), stop=(last and j == 1),
                        skip_group_check=True,
                    )

            def emit_p1_pair(s, i):
                nonlocal gdt
                xq = xqs[s]
                if i == 0:
                    mwall[s] = mwp.tile(
                        [128, 2, NP, 2, 128], F16, tag="mw", name=f"mw{s}"
                    )
                    if s == 0:
                        gdt = pG.tile([128, 2, 128], F32, tag="gd", name="gd")
                p1 = pP.tile([128, 1024], F32)
                p1v = p1.rearrange("p (j r c) -> p j r c", j=2, c=128)
                for j in (0, 1):
                    nt = 2 * i + j
                    e, col = nt // 8, (nt % 8) * 128
                    dst = p1[:, j * 512:j * 512 + 384]
                    for k in range(KC):
                        nc.tensor.matmul(
                            dst, xq[e][:, k, col:col + 128], wcat[:, k, :],
                            start=(k == 0),
                            stop=(k == KC - 1 and not has_bias_abv),
                        )
                    if has_bias_abv:
                        nc.tensor.matmul(
                            dst, ones1[:], bcat[:], start=False, stop=True
                        )

                # gd for pair i-4: keeps PE busy while softmax(i-3..i) runs
                if i >= 4:
                    emit_gd(s, i - 4, first=(i == 4), last=False)

                # exp(logit + SHIFT) -> fp16, straight into the mwall store
                # (dst strides permuted to the bv-major layout)
                nc.scalar.activation(
                    mwall[s][:, :, i, :, :].rearrange("p bv j c -> p j bv c"),
                    p1v[:, :, 1:3, :], EXP, bias=shift[:],
                )
                # A evacuated as a PLAIN cast, immediately after the matmuls:
                # releases the PSUM pair tile without waiting for the softmax
                # reduce/recip chain (1/sum is folded into the M half below)
                at = atp.tile([128, 2, 128], F16)
                nc.scalar.copy(at[:], p1v[:, :, 0, :])
                ats[s, i] = at
                # fp16 sums/recip on DVE; sums layout (bv, j)
                sums = stp.tile([128, 2, 2], F16, tag="sums")
                with nc.allow_low_precision(reason="128-term fp16 softmax sum"):
                    nc.vector.reduce_sum(sums[:], mwall[s][:, :, i, :, :],
                                         axis=mybir.AxisListType.X)
                rec = stp.tile([128, 2, 2], F16, tag="rec")
                with nc.allow_low_precision(reason="fp16 softmax 1/sum"):
                    nc.vector.reciprocal(rec[:], sums[:])
                # normalizations in place: M absorbs 1/sum(expB) (the
                # entire B normalization) on DVE right after recip -- no
                # cross-engine hop on the gd-critical chain; W 1/sum(expV)
                # on the otherwise idle POOL
                nc.vector.tensor_mul(
                    mwall[s][:, 0, i], mwall[s][:, 0, i],
                    rec[:, 0:1, :].rearrange("p o j -> p j o")
                    .broadcast_to([128, 2, 128]),
                )
                nc.gpsimd.tensor_mul(
                    mwall[s][:, 1, i], mwall[s][:, 1, i],
                    rec[:, 1:2, :].rearrange("p o j -> p j o")
                    .broadcast_to([128, 2, 128]),
                )

            def emit_tr(s, g, dma=False):
                wsb = wsbp.tile([128, 512], F16, tag="wsb", name=f"wsb{s}_{g}")
                if dma:
                    # W^T via the XBAR DMA transpose on the idle SP queue.
                    # The deadlock guard serializes it against outstanding
                    # DMAs, which is harmless mid-kernel (loads done, store
                    # traffic slack) and it takes the transpose + evacuation
                    # off the PE/DVE entirely.
                    nc.sync.dma_start(
                        wsb[:].rearrange("p (t q) -> p t q", t=KC),
                        mwall[s][:, 1, 2 * g:2 * g + 2, :, :]
                        .rearrange("p a j c -> p (a j c)"),
                        transpose=True,
                    )
                else:
                    # PE transpose into PSUM + DVE evacuation: real filler
                    # work that keeps the HAM clock warm through the
                    # end-of-P1 softmax drain
                    wpt = pW.tile(
                        [128, 512], F16, tag="wpt", name=f"wpt{s}_{g}"
                    )
                    for t in range(KC):
                        nt = 4 * g + t
                        nc.tensor.matmul(
                            wpt[:, t * 128:(t + 1) * 128],
                            mwall[s][:, 1, nt // 2, nt % 2, :],
                            ident16[:], is_transpose=True,
                            skip_group_check=True,
                        )
                    nc.vector.tensor_copy(wsb[:], wpt[:])
                wsbs[s, g] = wsb

            def emit_gd_tail(s):
                for i in range(NP - 4, NP):
                    emit_gd(s, i, first=False, last=(i == NP - 1))

            def emit_gt(s):
                gdts = gdsp.tile([128, 128], F16, tag="gdts", name=f"gdts{s}")
                nc.vector.tensor_copy(gdts[:], gdt[:, s, :])
                # G^T[k, c] = sum_m gd[m,k] wR[c,m] : one N=512 matmul
                gtp = pO.tile([128, 512], F32, tag="po", name=f"gtp{s}")
                nc.tensor.matmul(gtp[:], gdts[:], wrt[:], start=True, stop=True)
                g = gtsp.tile([128, 512], F16, tag="gts", name=f"gts{s}")
                nc.scalar.copy(g[:], gtp[:])
                gts[s] = g

            def emit_out_group(s, g):
                last_s = s == B_LOC - 1
                split = last_s and g >= NG - 2   # per-k store pieces

                dsto = out_d[s, :, g]            # [128, KC, 512], contiguous
                osb_t = osbp.tile(
                    [128, KC, 512], F16, tag="osb", name=f"osb{s}_{g}"
                )
                wsb_f = wsbs[s, g][:]

                def piece(ks, eng):
                    # store piece issued right after its evacuation lands,
                    # alternating issuing engines so the drain overlaps the
                    # remaining evacuations
                    eng.dma_start(
                        dsto[:, ks[0]:ks[-1] + 1, :],
                        osb_t[:, ks[0]:ks[-1] + 1, :],
                    )

                if last_s and not has_bias_r:
                    # k0,k1 through a 2-bank tile of the (idle by now) pP
                    # pool: deeper PSUM buffering for the tail-sample P3,
                    # and one wide ACT evacuation instead of two
                    op = pP.tile([128, 1024], F32, tag="p1", name=f"obk{s}_{g}")
                    opv = op.rearrange("p (k c) -> p k c", k=2)
                    for dk in (0, 1):
                        nc.tensor.matmul(
                            opv[:, dk, :], gts[s][:, dk * 128:(dk + 1) * 128],
                            wsb_f, start=True, stop=True,
                        )
                    nc.scalar.copy(osb_t[:, 0:2, :], opv[:])
                    if split:
                        piece((0, 1), nc.scalar)
                    ks = (2, 3)
                else:
                    ks = range(KC)
                for k in ks:
                    ock = pO.tile(
                        [128, 512], F32, tag="po", name=f"ock{s}_{g}_{k}"
                    )
                    nc.tensor.matmul(
                        ock[:], gts[s][:, k * 128:(k + 1) * 128],
                        wsb_f, start=True, stop=True,
                    )
                    dst = osb_t[:, k, :]
                    if has_bias_r:
                        nc.scalar.activation(
                            dst, ock[:], IDENT, bias=brt[:, k:k + 1]
                        )
                    elif not last_s and k <= 1:
                        nc.scalar.copy(dst, ock[:])
                    else:
                        nc.vector.tensor_copy(dst, ock[:])
                    if split:
                        piece((k,), nc.sync if k == 2 else nc.scalar)
                if not split:
                    # s0's stores issue from POOL's software DGE (loads own
                    # the HW DGEs then); s1's tail stores use the idle SP
                    # HWDGE -- its data path is ~2x faster and the loads are
                    # long finished
                    (nc.sync if last_s else nc.gpsimd).dma_start(
                        dsto[:], osb_t[:]
                    )

            # ---- schedule ----
            # s0 P1; s1's first two pairs fill s0's gd-tail chain; six of
            # s0's out-groups interleave with s1's P1 pairs and the LAST
            # TWO are held back as real PE work bridging s1's end-of-P1
            # softmax/gd/G^T chain (keeps the HAM clock gate at 8/8 into
            # the tail); s1's transposes spread across the pairs.
            for i in range(NP):
                emit_p1_pair(0, i)
            emit_p1_pair(1, 0)
            emit_p1_pair(1, 1)
            emit_gd_tail(0)
            emit_gt(0)
            emit_tr(0, 0)
            tr_sched = {10: (0,), 11: (1,), 12: (2,), 13: (3,)}
            og = 0
            for i in range(2, NP):
                emit_p1_pair(1, i)
                for g in tr_sched.get(i, ()):
                    emit_tr(1, g)
                if i % 2 == 0 and og < NG - 2:
                    emit_tr(0, og + 1)
                    emit_out_group(0, og)
                    og += 1
            emit_tr(1, 4)
            emit_tr(1, 5)
            emit_tr(0, 7)
            emit_out_group(0, 6)
            emit_gd_tail(1)
            emit_tr(1, 6)
            emit_out_group(0, 7)
            emit_gt(1)
            emit_tr(1, 7)
            for g in range(NG):
                emit_out_group(1, g)

    nc.compile()
    return nc


_CACHE = {}


def _get_nc(has_bias_abv: bool, has_bias_r: bool):
    key = (has_bias_abv, has_bias_r)
    if key not in _CACHE:
        _CACHE[key] = _build(*key)
    return _CACHE[key]


def _run(inputs, trace=False, **spmd_kwargs):
    x = np.asarray(inputs["x"])
    b, c, h, w = x.shape
    assert (b, c, h, w) == (B_GLOBAL, C_IN, H, W), x.shape
    wA = np.asarray(inputs["wA"], np.float32)
    wB = np.asarray(inputs["wB"], np.float32)
    wV = np.asarray(inputs["wV"], np.float32)
    wR = np.asarray(inputs["wR"], np.float32)
    bA = np.asarray(inputs["bA"], np.float32)
    bB = np.asarray(inputs["bB"], np.float32)
    bV = np.asarray(inputs["bV"], np.float32)
    bR = np.asarray(inputs["bR"], np.float32)

    has_bias_abv = bool(np.any(bA) or np.any(bB) or np.any(bV))
    has_bias_r = bool(np.any(bR))
    nc = _get_nc(has_bias_abv, has_bias_r)

    # [128, KC*384] : partition p, block k holds [wA.T|wB.T|wV.T][k*128+p, :]
    wcat = (np.concatenate([wA.T, wB.T, wV.T], axis=1)
            .reshape(KC, 128, 3 * 128).transpose(1, 0, 2).reshape(128, KC * 384))
    base = {
        "wcat": np.ascontiguousarray(wcat, dtype=np.float16),
        "wrt": np.ascontiguousarray(wR.T, dtype=np.float16),
    }
    if has_bias_abv:
        base["bcat"] = np.concatenate([bA, bB, bV])[None, :].astype(np.float16)
    if has_bias_r:
        base["brt"] = np.ascontiguousarray(bR.reshape(KC, 128).T, np.float32)

    # pack x to [b, 128, NE, KC, 1024]: partition p of slice e holds the
    # e-th 1024-col block of channels k*128+p for k=0..KC-1, so one slice
    # is a single 8KB-contiguous-per-partition DMA transfer
    xf = (np.asarray(x, np.float16).reshape(B_GLOBAL, KC, 128, NE, 1024)
          .transpose(0, 2, 3, 1, 4))
    in_maps = [
        dict(base, x=np.ascontiguousarray(xf[ci * B_LOC:(ci + 1) * B_LOC]))
        for ci in range(N_CORES)
    ]
    res = run_bass_kernel_spmd(
        nc, in_maps, core_ids=list(range(N_CORES)), trace=trace, **spmd_kwargs
    )
    out = np.concatenate(
        [res.results[ci]["out"] for ci in range(N_CORES)], axis=0
    )
    # unpack [b, 128, NG, KC, 512] -> [b, C_IN, N]: channel = k*128+p,
    # n = g*512+col
    out = (out.transpose(0, 3, 1, 2, 4).astype(np.float32)
           .reshape(B_GLOBAL, C_IN, H, W))
    return out, res


def kernel(**inputs):
    out, _ = _run(inputs)
    return out
